# revision 55
# baseline (speedup 1.0000x reference)
"""MicrotubuleAttention TRN2 kernel: head-sharded across 8 NeuronCores.

Core c handles q-heads {2c, 2c+1} and kv-head c//2.  Host prepares per-core
weight shards + RoPE tables; each core computes QKV projections, RoPE,
bias-augmented causal attention and its slice of the output projection.
Host sums the 8 partial output projections (the all-reduce).

Math notes:
 * 1/sqrt(d) is folded into Wq on the host.
 * Scores are computed TRANSPOSED: S^T[j, ti] via lhsT=K-tile so the exp
   output feeds the AV matmul directly as its stationary operand — no
   per-tile PE transposes of the probability matrix.
 * Softmax skips max-subtraction.  The (clip(pol)/4096 + softplus-gamma)
   decay bias enters the score PSUM as a rank-1 matmul: lhsT holds a
   3-term bf16 hi/mid/lo decomposition of c_h*(j - K_i) per head (K=6
   contraction), rhs is a constant head-selector.  With K_i = 128*i + 127
   all exponents stay <= qk + gate; the per-row constant cancels after
   division by the row sum.  gate*sigmoid(A.B) enters the same PSUM bank
   via a diag(gate) matmul.  Exp then runs bias-free over 4 j-tiles at
   once (amortizing the Act-engine access latency).
 * V carries an extra ones column so the AV matmul accumulates the
   softmax row-sum in the same PSUM accumulation group (PSUM zero-regions
   are bank-granular: two interleaved groups in one bank are unsafe).
 * sigmoid(A_i.B_j) for the whole causal triangle is computed in its own
   phase (one act-table load); the attention loop then only runs Exp.
"""
import numpy as np

D_MODEL = 2048
N_HEADS = 16
D_HEAD = 128
MAX_SEQ_LEN = 4096
RANK = 32
ROPE_BASE = 10000.0
T = 2048
N_CORES = 8
HPC = N_HEADS // N_CORES          # q heads per core = 2
P = 128
NT = T // P                       # 16
ND = D_MODEL // P                 # 16
NTRI = NT * (NT + 1) // 2         # 136 causal tiles
JG = 4                            # j-tiles per exp group


def _build_kernel():
    import concourse.bass as bass
    import concourse.mybir as mybir
    import concourse.tile as tile
    from concourse import bacc
    from concourse.masks import make_identity
    from contextlib import ExitStack

    f32 = mybir.dt.float32
    bf16 = mybir.dt.bfloat16
    AF = mybir.ActivationFunctionType
    ALU = mybir.AluOpType

    nc = bacc.Bacc("TRN2", target_bir_lowering=False, debug=False,
                   num_devices=N_CORES)

    xT = nc.dram_tensor("xT", [D_MODEL, T], bf16, kind="ExternalInput")
    # [q0 q1 k v] columns, 1/sqrt(d) folded into q part
    wqkv = nc.dram_tensor("wqkv", [D_MODEL, 512], bf16, kind="ExternalInput")
    wab = nc.dram_tensor("wab", [D_MODEL, 2 * RANK], bf16, kind="ExternalInput")
    wo = nc.dram_tensor("wo", [HPC * D_HEAD, D_MODEL], bf16, kind="ExternalInput")
    cs = nc.dram_tensor("cs", [T, 2 * D_HEAD], bf16, kind="ExternalInput")
    # hpar[0, h] = gate_h
    hpar = nc.dram_tensor("hpar", [1, HPC], f32, kind="ExternalInput")
    # cjtab[k, d*128 + j] = bf16 component k%3 of c_{k//3}*(j - 127 - 128*d)
    cjtab = nc.dram_tensor("cjtab", [3 * HPC, NT * P], bf16, kind="ExternalInput")
    seld = nc.dram_tensor("seld", [3 * HPC, HPC * P], bf16, kind="ExternalInput")
    out = nc.dram_tensor("out", [T, D_MODEL], f32, kind="ExternalOutput")

    KC = 3 * HPC   # rank-1 bias contraction size

    with tile.TileContext(nc) as tc, ExitStack() as ctx:
        singles = ctx.enter_context(tc.tile_pool(name="singles", bufs=1))
        work = ctx.enter_context(tc.tile_pool(name="work", bufs=3))
        small = ctx.enter_context(tc.tile_pool(name="small", bufs=4))

        ident = singles.tile([P, P], bf16)
        make_identity(nc, ident)

        gates = singles.tile([P, HPC], f32)
        hap = hpar[:]
        nc.sync.dma_start(
            out=gates[:],
            in_=bass.AP(tensor=hap.tensor, offset=hap.offset,
                        ap=[[0, P], hap.ap[1]]))
        # diagI[:, h] = gate_h * I: accumulates gate*sigmoid into score PSUM
        diagI = singles.tile([P, HPC, P], bf16)
        for h in range(HPC):
            nc.gpsimd.tensor_scalar_mul(diagI[:, h], ident[:], gates[:, h:h + 1])
        # head-selector for the rank-1 bias matmul: sel[3h+k, h*128:(h+1)*128]=1
        sel = singles.tile([P, HPC * P], bf16)
        nc.sync.dma_start(out=sel[0:KC, :], in_=seld[:, :])
        cj_sb = singles.tile([P, NT, P], bf16)       # rows 0:KC used
        nc.sync.dma_start(out=cj_sb[0:KC, :, :], in_=cjtab[:, :])

        qt_sb = singles.tile([P, HPC, T], bf16)      # Q^T per head [d, t]
        kt_sb = singles.tile([P, T], bf16)           # K^T [d, t]
        # V tiles [t, d] augmented with a ones column (col 128) so the AV
        # matmul also accumulates the softmax row-sum in one PSUM group
        v_sb = singles.tile([P, NT, D_HEAD + 1], bf16)
        nc.gpsimd.memset(v_sb[:, :, D_HEAD:D_HEAD + 1], 1.0)
        at_sb = singles.tile([P, T], bf16)           # rows 0:32 = A^T [r, t]
        bt_sb = singles.tile([P, T], bf16)           # rows 0:32 = B^T [r, t]
        msig = singles.tile([P, NTRI, P], bf16)      # sigmoid(A_i.B_j) [j, ti]
        aoutT = singles.tile([P, HPC, T], bf16)      # attn-out^T [d, t] per head
        wo_sb = singles.tile([P, HPC, D_MODEL], bf16)

        # ---------------- projections + RoPE (scoped SBUF) ----------------
        with tc.tile_pool(name="proj", bufs=1) as proj, \
             tc.tile_pool(name="ppq", bufs=4, space="PSUM") as ppq, \
             tc.tile_pool(name="ppt", bufs=2, space="PSUM") as ppt, \
             tc.tile_pool(name="pms", bufs=2, space="PSUM") as pms:
            xT_sb = proj.tile([P, ND, T], bf16)
            wqkv_sb = proj.tile([P, ND, 512], bf16)
            wab_sb = proj.tile([P, ND, 2 * RANK], bf16)
            cs_sb = proj.tile([P, NT, 2 * D_HEAD], bf16)
            # interleave xT / weight chunk loads so the d-outer QKV matmul
            # chain can start as soon as the first chunks land
            for d in range(ND):
                sl = slice(d * P, (d + 1) * P)
                nc.sync.dma_start(out=xT_sb[:, d], in_=xT[sl, :])
                nc.sync.dma_start(out=wqkv_sb[:, d], in_=wqkv[sl, :])
                if d < 8:
                    i2 = 2 * d
                    nc.sync.dma_start(out=cs_sb[:, i2],
                                      in_=cs[i2 * P:(i2 + 1) * P, :])
                    nc.sync.dma_start(out=cs_sb[:, i2 + 1],
                                      in_=cs[(i2 + 1) * P:(i2 + 2) * P, :])
            for d in range(ND):
                sl = slice(d * P, (d + 1) * P)
                nc.sync.dma_start(out=wab_sb[:, d], in_=wab[sl, :])
            for h in range(HPC):
                nc.sync.dma_start(out=wo_sb[:, h], in_=wo[h * P:(h + 1) * P, :])

            rope_pending = []
            ab_pending = []

            def emit_rope_t(rtsl, rroped):
                ptr = ppt.tile([P, 3, P], bf16, tag="pt")
                for hh in range(3):      # q0, q1, k
                    nc.tensor.transpose(ptr[:, hh, :], rroped[:, hh, :],
                                        ident[:])
                    dst = qt_sb[:, hh, rtsl] if hh < HPC else kt_sb[:, rtsl]
                    nc.vector.tensor_copy(dst, ptr[:, hh, :])

            def emit_ab_t(rtsl, rpabs):
                ptab = ppt.tile([P, 3, P], bf16, tag="pt")
                nc.tensor.transpose(ptab[0:RANK, 0, :], rpabs[:, 0:RANK],
                                    ident[:])
                nc.tensor.transpose(ptab[0:RANK, 1, :], rpabs[:, RANK:2 * RANK],
                                    ident[:])
                nc.vector.tensor_copy(at_sb[0:RANK, rtsl], ptab[0:RANK, 0, :])
                nc.vector.tensor_copy(bt_sb[0:RANK, rtsl], ptab[0:RANK, 1, :])

            for g in [0]:
                pq0 = ppq.tile([P, 512], f32, tag="pq")
                pq1 = ppq.tile([P, 512], f32, tag="pq")
                pq2 = ppq.tile([P, 512], f32, tag="pq")
                pq3 = ppq.tile([P, 512], f32, tag="pq")
                pqs = [pq0, pq1, pq2, pq3]
                for d in range(ND):
                    for k in range(4):
                        i = 4 * g + k
                        tsl = slice(i * P, (i + 1) * P)
                        nc.tensor.matmul(pqs[k][:], xT_sb[:, d, tsl],
                                         wqkv_sb[:, d],
                                         start=(d == 0), stop=(d == ND - 1))
                for k in range(4):
                    i = 4 * g + k
                    tsl = slice(i * P, (i + 1) * P)
                    pq = pqs[k]
                    nc.scalar.copy(v_sb[:, i, 0:D_HEAD], pq[:, 384:512])
                    qks = work.tile([P, 3, D_HEAD], f32, tag="qks")
                    nc.scalar.copy(qks[:], pq[:, 0:3 * D_HEAD])
                    # rotate-half across all 3 heads at once (Pool, SBUF only)
                    rot = work.tile([P, 3, D_HEAD], f32, tag="rot")
                    nc.gpsimd.tensor_scalar_mul(rot[:, :, 0:64],
                                                qks[:, :, 64:128], -1.0)
                    nc.gpsimd.tensor_copy(rot[:, :, 64:128], qks[:, :, 0:64])
                    ca = cs_sb[:, i, 0:D_HEAD]
                    cos3 = bass.AP(tensor=ca.tensor, offset=ca.offset,
                                   ap=[ca.ap[0], [0, 3], [1, D_HEAD]])
                    sa = cs_sb[:, i, D_HEAD:2 * D_HEAD]
                    sin3 = bass.AP(tensor=sa.tensor, offset=sa.offset,
                                   ap=[sa.ap[0], [0, 3], [1, D_HEAD]])
                    m1 = work.tile([P, 3, D_HEAD], f32, tag="m1")
                    nc.vector.tensor_mul(m1[:], qks[:], cos3)
                    m2 = work.tile([P, 3, D_HEAD], f32, tag="m2")
                    nc.vector.tensor_mul(m2[:], rot[:], sin3)
                    roped = work.tile([P, 3, D_HEAD], bf16, tag="roped")
                    nc.vector.tensor_add(roped[:], m1[:], m2[:])
                    if rope_pending:
                        emit_rope_t(*rope_pending.pop(0))
                    rope_pending.append((tsl, roped))
            # A|B in [t, 64] layout (full-partition matmuls), then PE
            # transposes back to [r, t] at partitions 0:32
            for i in range(NT):
                tsl = slice(i * P, (i + 1) * P)
                pab = ppq.tile([P, 512], f32, tag="pq")
                for d in range(ND):
                    nc.tensor.matmul(pab[:, 0:2 * RANK], xT_sb[:, d, tsl],
                                     wab_sb[:, d],
                                     start=(d == 0), stop=(d == ND - 1))
                pabs = work.tile([P, 2 * RANK], bf16, tag="pabs")
                nc.scalar.copy(pabs[:], pab[:, 0:2 * RANK])
                if ab_pending:
                    emit_ab_t(*ab_pending.pop(0))
                ab_pending.append((tsl, pabs))

            for pnd in ab_pending:
                emit_ab_t(*pnd)
            ab_pending.clear()
            # ---- sigmoid triangle: interleaved into the QKV d-steps below
            # (dedicated pms pool so Act-paced draining never blocks ppq) ----
            sig_tasks = [(i, g0, min(4, i + 1 - g0))
                         for i in range(NT) for g0 in range(0, i + 1, 4)]
            sig_pos = [0]

            def emit_sig(n):
                for si, sg0, sgn in sig_tasks[sig_pos[0]:sig_pos[0] + n]:
                    s_isl = slice(si * P, (si + 1) * P)
                    sbase = si * (si + 1) // 2
                    mp = pms.tile([P, 512], f32, tag="mp")
                    for sk in range(sgn):
                        jsl = slice((sg0 + sk) * P, (sg0 + sk + 1) * P)
                        nc.tensor.matmul(mp[:, sk * P:(sk + 1) * P],
                                         bt_sb[0:RANK, jsl],
                                         at_sb[0:RANK, s_isl],
                                         start=True, stop=True)
                    nc.scalar.activation(
                        msig[:, sbase + sg0:sbase + sg0 + sgn, :],
                        mp[:, 0:sgn * P], AF.Sigmoid)
                sig_pos[0] += n

            # ---- remaining QKV groups ----
            for g in [1, 2, 3]:
                pq0 = ppq.tile([P, 512], f32, tag="pq")
                pq1 = ppq.tile([P, 512], f32, tag="pq")
                pq2 = ppq.tile([P, 512], f32, tag="pq")
                pq3 = ppq.tile([P, 512], f32, tag="pq")
                pqs = [pq0, pq1, pq2, pq3]
                for d in range(ND):
                    for k in range(4):
                        i = 4 * g + k
                        tsl = slice(i * P, (i + 1) * P)
                        nc.tensor.matmul(pqs[k][:], xT_sb[:, d, tsl],
                                         wqkv_sb[:, d],
                                         start=(d == 0), stop=(d == ND - 1))
                    emit_sig(1)
                for k in range(4):
                    i = 4 * g + k
                    tsl = slice(i * P, (i + 1) * P)
                    pq = pqs[k]
                    nc.scalar.copy(v_sb[:, i, 0:D_HEAD], pq[:, 384:512])
                    qks = work.tile([P, 3, D_HEAD], f32, tag="qks")
                    nc.scalar.copy(qks[:], pq[:, 0:3 * D_HEAD])
                    rot = work.tile([P, 3, D_HEAD], f32, tag="rot")
                    nc.gpsimd.tensor_scalar_mul(rot[:, :, 0:64],
                                                qks[:, :, 64:128], -1.0)
                    nc.gpsimd.tensor_copy(rot[:, :, 64:128], qks[:, :, 0:64])
                    ca = cs_sb[:, i, 0:D_HEAD]
                    cos3 = bass.AP(tensor=ca.tensor, offset=ca.offset,
                                   ap=[ca.ap[0], [0, 3], [1, D_HEAD]])
                    sa = cs_sb[:, i, D_HEAD:2 * D_HEAD]
                    sin3 = bass.AP(tensor=sa.tensor, offset=sa.offset,
                                   ap=[sa.ap[0], [0, 3], [1, D_HEAD]])
                    m1 = work.tile([P, 3, D_HEAD], f32, tag="m1")
                    nc.vector.tensor_mul(m1[:], qks[:], cos3)
                    m2 = work.tile([P, 3, D_HEAD], f32, tag="m2")
                    nc.vector.tensor_mul(m2[:], rot[:], sin3)
                    roped = work.tile([P, 3, D_HEAD], bf16, tag="roped")
                    nc.vector.tensor_add(roped[:], m1[:], m2[:])
                    if rope_pending:
                        emit_rope_t(*rope_pending.pop(0))
                    rope_pending.append((tsl, roped))
            emit_sig(len(sig_tasks) - sig_pos[0])
            for pnd in rope_pending:
                emit_rope_t(*pnd)
            rope_pending.clear()

        # ---------------- attention ----------------
        with tc.tile_pool(name="psS", bufs=2, space="PSUM") as psS, \
             tc.tile_pool(name="psAV", bufs=2, space="PSUM") as psAV, \
             tc.tile_pool(name="psT", bufs=1, space="PSUM") as psT, \
             tc.tile_pool(name="psPO", bufs=1, space="PSUM") as psPO, \
             tc.tile_pool(name="pmp", bufs=4) as pmp, \
             tc.tile_pool(name="outp", bufs=4) as outp:
            pending = []

            def emit_row_tail(ri, risl, raosbs):
                for h in range(HPC):
                    pt = psT.tile([P, P], bf16, tag="pt")
                    nc.tensor.transpose(pt[:], raosbs[h][:], ident[:])
                    nc.vector.tensor_copy(aoutT[:, h, risl], pt[:])
                for mch in range(D_MODEL // 512):
                    po = psPO.tile([P, 512], f32, tag="po")
                    for h in range(HPC):
                        nc.tensor.matmul(po[:], aoutT[:, h, risl],
                                         wo_sb[:, h, mch * 512:(mch + 1) * 512],
                                         start=(h == 0), stop=(h == HPC - 1))
                    ob = outp.tile([P, 512], f32, tag="ob")
                    if mch % 2 == 0:
                        nc.scalar.copy(ob[:], po[:])
                    else:
                        nc.vector.tensor_copy(ob[:], po[:])
                    nc.sync.dma_start(
                        out=out[risl, mch * 512:(mch + 1) * 512], in_=ob[:])

            for i in range(NT):
                isl = slice(i * P, (i + 1) * P)
                base = i * (i + 1) // 2
                avrs0 = psAV.tile([P, P + 1], f32, tag="avrs")
                avrs1 = psAV.tile([P, P + 1], f32, tag="avrs")
                avrs = [avrs0, avrs1]
                for g0 in range(0, i + 1, JG):
                    gn = min(JG, i + 1 - g0)
                    spg = psS.tile([P, JG, HPC * P], f32, tag="spg")
                    for k in range(gn):
                        jt = g0 + k
                        jsl = slice(jt * P, (jt + 1) * P)
                        nc.tensor.matmul(spg[:, k, :], kt_sb[:, jsl],
                                         qt_sb[:, :, isl],
                                         start=True, stop=False)
                        for h in range(HPC):
                            nc.tensor.matmul(spg[:, k, h * P:(h + 1) * P],
                                             diagI[:, h], msig[:, base + jt, :],
                                             start=False, stop=False)
                        # decay bias c_h*(j - K_i): rank-1 over the 6-row
                        # hi/mid/lo-per-head decomposition
                        nc.tensor.matmul(spg[:, k, :], cj_sb[0:KC, i - jt, :],
                                         sel[0:KC, :],
                                         start=False, stop=True)
                    pmg = pmp.tile([P, HPC, JG, P], bf16, tag="pmg")
                    # one exp for both heads: out AP iterates (slot, h, col)
                    # to match the spg free-dim order
                    pa = pmg[:, 0, 0, :]
                    pmga = bass.AP(
                        tensor=pa.tensor, offset=pa.offset,
                        ap=[pa.ap[0], [P, gn], [JG * P, HPC], [1, P]])
                    nc.scalar.activation(pmga, spg[:, 0:gn, :], AF.Exp)
                    for h in range(HPC):
                        if g0 + gn - 1 == i:
                            # causal diag tile: keep ti (free) >= j, else 0
                            nc.gpsimd.affine_select(
                                out=pmg[:, h, gn - 1, :],
                                in_=pmg[:, h, gn - 1, :], pattern=[[1, P]],
                                compare_op=ALU.is_ge, fill=0.0,
                                base=0, channel_multiplier=-1)
                        for k in range(gn):
                            jt = g0 + k
                            nc.tensor.matmul(avrs[h][:, 0:P + 1],
                                             pmg[:, h, k, :], v_sb[:, jt],
                                             start=(jt == 0), stop=(jt == i))
                # reciprocal + scale on DVE; PE-side transpose + out-proj
                # are DEFERRED one row so the in-order PE queue never stalls
                # on this DVE chain
                aosbs = []
                for h in range(HPC):
                    rinv = small.tile([P, 1], f32, tag="rinv")
                    nc.vector.reciprocal(rinv[:], avrs[h][:, P:P + 1])
                    aosb = work.tile([P, P], bf16, tag="aosb")
                    nc.vector.tensor_scalar_mul(aosb[:], avrs[h][:, 0:P], rinv[:])
                    aosbs.append(aosb)
                if pending:
                    emit_row_tail(*pending[0])
                    pending.clear()
                pending.append((i, isl, aosbs))
            emit_row_tail(*pending[0])
    nc.compile()
    return nc


_NC_CACHE = None


def kernel(**inputs):
    global _NC_CACHE
    x = np.asarray(inputs["x"])
    Wq = np.asarray(inputs["Wq"]); Wk = np.asarray(inputs["Wk"])
    Wv = np.asarray(inputs["Wv"]); Wo = np.asarray(inputs["Wo"])
    pol_dir = np.asarray(inputs["pol_dir"]); pol_WA = np.asarray(inputs["pol_WA"])
    pol_WB = np.asarray(inputs["pol_WB"]); pol_gate = np.asarray(inputs["pol_gate"])
    gtp_gamma = np.asarray(inputs["gtp_gamma"])

    import ml_dtypes
    bf = ml_dtypes.bfloat16
    assert x.shape == (1, T, D_MODEL)

    pol = np.clip(pol_dir.astype(np.float64), -1.0, 1.0)
    gamma = np.maximum(np.log1p(np.exp(gtp_gamma.astype(np.float64))), 1e-6)
    c_h = (pol / float(MAX_SEQ_LEN) + gamma).astype(np.float64)
    gate = (1.0 / (1.0 + np.exp(-pol_gate.astype(np.float64)))).astype(np.float32)

    inv_freq = 1.0 / (ROPE_BASE ** (np.arange(0, D_HEAD, 2, dtype=np.float64) / D_HEAD))
    ang = np.arange(T, dtype=np.float64)[:, None] * inv_freq[None, :]
    cosf = np.concatenate([np.cos(ang), np.cos(ang)], 1)
    sinf = np.concatenate([np.sin(ang), np.sin(ang)], 1)
    csf = np.concatenate([cosf, sinf], 1).astype(bf)   # [T, 256]

    xT = np.ascontiguousarray(x[0].T).astype(bf)
    inv_sqrt_d = 1.0 / np.sqrt(float(D_HEAD))

    # cj[p, d] = c_h * (p - 127 - 128*d) as 3-term bf16 decomposition
    pcol = np.arange(P, dtype=np.float64)[None, :]          # [1, P] (j within tile)
    dgrid = np.arange(NT, dtype=np.float64)[:, None]        # [NT, 1]
    off = pcol - 127.0 - 128.0 * dgrid                      # [NT, P]

    sel_np = np.zeros((3 * HPC, HPC * P), dtype=bf)
    for h in range(HPC):
        sel_np[3 * h:3 * h + 3, h * P:(h + 1) * P] = bf(1.0)

    in_maps = []
    for c in range(N_CORES):
        hs = slice(2 * c * D_HEAD, (2 * c + 2) * D_HEAD)
        kvh = c // 2
        kv = slice(kvh * D_HEAD, (kvh + 1) * D_HEAD)
        wqkv_np = np.concatenate(
            [Wq[:, hs] * inv_sqrt_d, Wk[:, kv], Wv[:, kv]], axis=1)
        cjtab_np = np.zeros((3 * HPC, NT * P), dtype=bf)
        for h in range(HPC):
            val = (c_h[2 * c + h] * off).reshape(-1)        # [NT*P]
            rem = val.copy()
            for kk in range(3):
                comp = rem.astype(bf)
                cjtab_np[3 * h + kk] = comp
                rem = rem - comp.astype(np.float64)
        in_maps.append({
            "xT": xT,
            "wqkv": np.ascontiguousarray(wqkv_np).astype(bf),
            "wab": np.ascontiguousarray(
                np.concatenate([pol_WA, pol_WB], axis=1)).astype(bf),
            "wo": np.ascontiguousarray(Wo[hs, :]).astype(bf),
            "cs": csf,
            "hpar": gate[None, 2 * c:2 * c + 2].astype(np.float32),
            "cjtab": cjtab_np,
            "seld": sel_np,
        })

    if _NC_CACHE is None:
        _NC_CACHE = _build_kernel()
    from concourse.bass_utils import run_bass_kernel_spmd
    res = run_bass_kernel_spmd(_NC_CACHE, in_maps, core_ids=list(range(N_CORES)))
    total = np.zeros((T, D_MODEL), dtype=np.float32)
    for c in range(N_CORES):
        total += res.results[c]["out"]
    return total[None, :, :]


# revision 56
# speedup vs baseline: 1.0014x; 1.0014x over previous
"""MicrotubuleAttention TRN2 kernel: head-sharded across 8 NeuronCores.

Core c handles q-heads {2c, 2c+1} and kv-head c//2.  Host prepares per-core
weight shards + RoPE tables; each core computes QKV projections, RoPE,
bias-augmented causal attention and its slice of the output projection.
Host sums the 8 partial output projections (the all-reduce).

Math notes:
 * 1/sqrt(d) is folded into Wq on the host.
 * Scores are computed TRANSPOSED: S^T[j, ti] via lhsT=K-tile so the exp
   output feeds the AV matmul directly as its stationary operand — no
   per-tile PE transposes of the probability matrix.
 * Softmax skips max-subtraction.  The (clip(pol)/4096 + softplus-gamma)
   decay bias enters the score PSUM as a rank-1 matmul: lhsT holds a
   3-term bf16 hi/mid/lo decomposition of c_h*(j - K_i) per head (K=6
   contraction), rhs is a constant head-selector.  With K_i = 128*i + 127
   all exponents stay <= qk + gate; the per-row constant cancels after
   division by the row sum.  gate*sigmoid(A.B) enters the same PSUM bank
   via a diag(gate) matmul.  Exp then runs bias-free over 4 j-tiles at
   once (amortizing the Act-engine access latency).
 * V carries an extra ones column so the AV matmul accumulates the
   softmax row-sum in the same PSUM accumulation group (PSUM zero-regions
   are bank-granular: two interleaved groups in one bank are unsafe).
 * sigmoid(A_i.B_j) for the whole causal triangle is computed in its own
   phase (one act-table load); the attention loop then only runs Exp.
"""
import numpy as np

D_MODEL = 2048
N_HEADS = 16
D_HEAD = 128
MAX_SEQ_LEN = 4096
RANK = 32
ROPE_BASE = 10000.0
T = 2048
N_CORES = 8
HPC = N_HEADS // N_CORES          # q heads per core = 2
P = 128
NT = T // P                       # 16
ND = D_MODEL // P                 # 16
NTRI = NT * (NT + 1) // 2         # 136 causal tiles
JG = 4                            # j-tiles per exp group


def _build_kernel():
    import concourse.bass as bass
    import concourse.mybir as mybir
    import concourse.tile as tile
    from concourse import bacc
    from concourse.masks import make_identity
    from contextlib import ExitStack

    f32 = mybir.dt.float32
    bf16 = mybir.dt.bfloat16
    AF = mybir.ActivationFunctionType
    ALU = mybir.AluOpType

    nc = bacc.Bacc("TRN2", target_bir_lowering=False, debug=False,
                   num_devices=N_CORES)

    xT = nc.dram_tensor("xT", [D_MODEL, T], bf16, kind="ExternalInput")
    # [q0 q1 k v] columns, 1/sqrt(d) folded into q part
    wqkv = nc.dram_tensor("wqkv", [D_MODEL, 512], bf16, kind="ExternalInput")
    wab = nc.dram_tensor("wab", [D_MODEL, 2 * RANK], bf16, kind="ExternalInput")
    wo = nc.dram_tensor("wo", [HPC * D_HEAD, D_MODEL], bf16, kind="ExternalInput")
    cs = nc.dram_tensor("cs", [T, 2 * D_HEAD], bf16, kind="ExternalInput")
    # hpar[0, h] = gate_h
    hpar = nc.dram_tensor("hpar", [1, HPC], f32, kind="ExternalInput")
    # cjtab[k, d*128 + j] = bf16 component k%3 of c_{k//3}*(j - 127 - 128*d)
    cjtab = nc.dram_tensor("cjtab", [3 * HPC, NT * P], bf16, kind="ExternalInput")
    seld = nc.dram_tensor("seld", [3 * HPC, HPC * P], bf16, kind="ExternalInput")
    out = nc.dram_tensor("out", [T, D_MODEL], f32, kind="ExternalOutput")

    KC = 3 * HPC   # rank-1 bias contraction size

    with tile.TileContext(nc) as tc, ExitStack() as ctx:
        singles = ctx.enter_context(tc.tile_pool(name="singles", bufs=1))
        work = ctx.enter_context(tc.tile_pool(name="work", bufs=3))
        small = ctx.enter_context(tc.tile_pool(name="small", bufs=4))

        ident = singles.tile([P, P], bf16)
        make_identity(nc, ident)

        gates = singles.tile([P, HPC], f32)
        hap = hpar[:]
        nc.sync.dma_start(
            out=gates[:],
            in_=bass.AP(tensor=hap.tensor, offset=hap.offset,
                        ap=[[0, P], hap.ap[1]]))
        # diagI[:, h] = gate_h * I: accumulates gate*sigmoid into score PSUM
        diagI = singles.tile([P, HPC, P], bf16)
        for h in range(HPC):
            nc.gpsimd.tensor_scalar_mul(diagI[:, h], ident[:], gates[:, h:h + 1])
        # head-selector for the rank-1 bias matmul: sel[3h+k, h*128:(h+1)*128]=1
        sel = singles.tile([P, HPC * P], bf16)
        nc.sync.dma_start(out=sel[0:KC, :], in_=seld[:, :])
        cj_sb = singles.tile([P, NT, P], bf16)       # rows 0:KC used
        nc.sync.dma_start(out=cj_sb[0:KC, :, :], in_=cjtab[:, :])

        qt_sb = singles.tile([P, HPC, T], bf16)      # Q^T per head [d, t]
        kt_sb = singles.tile([P, T], bf16)           # K^T [d, t]
        # V tiles [t, d] augmented with a ones column (col 128) so the AV
        # matmul also accumulates the softmax row-sum in one PSUM group
        v_sb = singles.tile([P, NT, D_HEAD + 1], bf16)
        nc.gpsimd.memset(v_sb[:, :, D_HEAD:D_HEAD + 1], 1.0)
        at_sb = singles.tile([P, T], bf16)           # rows 0:32 = A^T [r, t]
        bt_sb = singles.tile([P, T], bf16)           # rows 0:32 = B^T [r, t]
        msig = singles.tile([P, NTRI, P], bf16)      # sigmoid(A_i.B_j) [j, ti]
        aoutT = singles.tile([P, HPC, T], bf16)      # attn-out^T [d, t] per head
        wo_sb = singles.tile([P, HPC, D_MODEL], bf16)

        # ---------------- projections + RoPE (scoped SBUF) ----------------
        with tc.tile_pool(name="proj", bufs=1) as proj, \
             tc.tile_pool(name="ppq", bufs=4, space="PSUM") as ppq, \
             tc.tile_pool(name="ppt", bufs=2, space="PSUM") as ppt, \
             tc.tile_pool(name="pms", bufs=2, space="PSUM") as pms:
            xT_sb = proj.tile([P, ND, T], bf16)
            wqkv_sb = proj.tile([P, ND, 512], bf16)
            wab_sb = proj.tile([P, ND, 2 * RANK], bf16)
            cs_sb = proj.tile([P, NT, 2 * D_HEAD], bf16)
            # interleave xT / weight chunk loads so the d-outer QKV matmul
            # chain can start as soon as the first chunks land
            for d in range(ND):
                sl = slice(d * P, (d + 1) * P)
                nc.sync.dma_start(out=xT_sb[:, d], in_=xT[sl, :])
                nc.sync.dma_start(out=wqkv_sb[:, d], in_=wqkv[sl, :])
                if d < 8:
                    i2 = 2 * d
                    nc.sync.dma_start(out=cs_sb[:, i2],
                                      in_=cs[i2 * P:(i2 + 1) * P, :])
                    nc.sync.dma_start(out=cs_sb[:, i2 + 1],
                                      in_=cs[(i2 + 1) * P:(i2 + 2) * P, :])
            for d in range(ND):
                sl = slice(d * P, (d + 1) * P)
                nc.sync.dma_start(out=wab_sb[:, d], in_=wab[sl, :])
            for h in range(HPC):
                nc.sync.dma_start(out=wo_sb[:, h], in_=wo[h * P:(h + 1) * P, :])

            rope_pending = []
            ab_pending = []

            def emit_rope_t(rtsl, rroped):
                ptr = ppt.tile([P, 3, P], bf16, tag="pt")
                for hh in range(3):      # q0, q1, k
                    nc.tensor.transpose(ptr[:, hh, :], rroped[:, hh, :],
                                        ident[:])
                    dst = qt_sb[:, hh, rtsl] if hh < HPC else kt_sb[:, rtsl]
                    nc.vector.tensor_copy(dst, ptr[:, hh, :])

            def emit_ab_t(rtsl, rpabs):
                ptab = ppt.tile([P, 3, P], bf16, tag="pt")
                nc.tensor.transpose(ptab[0:RANK, 0, :], rpabs[:, 0:RANK],
                                    ident[:])
                nc.tensor.transpose(ptab[0:RANK, 1, :], rpabs[:, RANK:2 * RANK],
                                    ident[:])
                nc.vector.tensor_copy(at_sb[0:RANK, rtsl], ptab[0:RANK, 0, :])
                nc.vector.tensor_copy(bt_sb[0:RANK, rtsl], ptab[0:RANK, 1, :])

            for g in [0]:
                pq0 = ppq.tile([P, 512], f32, tag="pq")
                pq1 = ppq.tile([P, 512], f32, tag="pq")
                pq2 = ppq.tile([P, 512], f32, tag="pq")
                pq3 = ppq.tile([P, 512], f32, tag="pq")
                pqs = [pq0, pq1, pq2, pq3]
                for d in range(ND):
                    for k in range(4):
                        i = 4 * g + k
                        tsl = slice(i * P, (i + 1) * P)
                        nc.tensor.matmul(pqs[k][:], xT_sb[:, d, tsl],
                                         wqkv_sb[:, d],
                                         start=(d == 0), stop=(d == ND - 1))
                for k in range(4):
                    i = 4 * g + k
                    tsl = slice(i * P, (i + 1) * P)
                    pq = pqs[k]
                    nc.scalar.copy(v_sb[:, i, 0:D_HEAD], pq[:, 384:512])
                    qks = work.tile([P, 3, D_HEAD], f32, tag="qks")
                    nc.scalar.copy(qks[:], pq[:, 0:3 * D_HEAD])
                    # rotate-half across all 3 heads at once (Pool, SBUF only)
                    rot = work.tile([P, 3, D_HEAD], f32, tag="rot")
                    nc.gpsimd.tensor_scalar_mul(rot[:, :, 0:64],
                                                qks[:, :, 64:128], -1.0)
                    nc.gpsimd.tensor_copy(rot[:, :, 64:128], qks[:, :, 0:64])
                    ca = cs_sb[:, i, 0:D_HEAD]
                    cos3 = bass.AP(tensor=ca.tensor, offset=ca.offset,
                                   ap=[ca.ap[0], [0, 3], [1, D_HEAD]])
                    sa = cs_sb[:, i, D_HEAD:2 * D_HEAD]
                    sin3 = bass.AP(tensor=sa.tensor, offset=sa.offset,
                                   ap=[sa.ap[0], [0, 3], [1, D_HEAD]])
                    m1 = work.tile([P, 3, D_HEAD], f32, tag="m1")
                    nc.vector.tensor_mul(m1[:], qks[:], cos3)
                    m2 = work.tile([P, 3, D_HEAD], f32, tag="m2")
                    nc.vector.tensor_mul(m2[:], rot[:], sin3)
                    roped = work.tile([P, 3, D_HEAD], bf16, tag="roped")
                    nc.vector.tensor_add(roped[:], m1[:], m2[:])
                    if rope_pending:
                        emit_rope_t(*rope_pending.pop(0))
                    rope_pending.append((tsl, roped))
            # A|B in [t, 64] layout (full-partition matmuls), then PE
            # transposes back to [r, t] at partitions 0:32
            for i in range(NT):
                tsl = slice(i * P, (i + 1) * P)
                pab = ppq.tile([P, 512], f32, tag="pq")
                for d in range(ND):
                    nc.tensor.matmul(pab[:, 0:2 * RANK], xT_sb[:, d, tsl],
                                     wab_sb[:, d],
                                     start=(d == 0), stop=(d == ND - 1))
                pabs = work.tile([P, 2 * RANK], bf16, tag="pabs")
                nc.scalar.copy(pabs[:], pab[:, 0:2 * RANK])
                if ab_pending:
                    emit_ab_t(*ab_pending.pop(0))
                ab_pending.append((tsl, pabs))

            for pnd in ab_pending:
                emit_ab_t(*pnd)
            ab_pending.clear()
            # ---- sigmoid triangle: interleaved into the QKV d-steps below
            # (dedicated pms pool so Act-paced draining never blocks ppq) ----
            sig_tasks = [(i, g0, min(4, i + 1 - g0))
                         for i in range(NT) for g0 in range(0, i + 1, 4)]
            sig_pos = [0]

            def emit_sig(n):
                for si, sg0, sgn in sig_tasks[sig_pos[0]:sig_pos[0] + n]:
                    s_isl = slice(si * P, (si + 1) * P)
                    sbase = si * (si + 1) // 2
                    mp = pms.tile([P, 512], f32, tag="mp")
                    for sk in range(sgn):
                        jsl = slice((sg0 + sk) * P, (sg0 + sk + 1) * P)
                        nc.tensor.matmul(mp[:, sk * P:(sk + 1) * P],
                                         bt_sb[0:RANK, jsl],
                                         at_sb[0:RANK, s_isl],
                                         start=True, stop=True)
                    nc.scalar.activation(
                        msig[:, sbase + sg0:sbase + sg0 + sgn, :],
                        mp[:, 0:sgn * P], AF.Sigmoid)
                sig_pos[0] += n

            # ---- remaining QKV groups ----
            for g in [1, 2, 3]:
                pq0 = ppq.tile([P, 512], f32, tag="pq")
                pq1 = ppq.tile([P, 512], f32, tag="pq")
                pq2 = ppq.tile([P, 512], f32, tag="pq")
                pq3 = ppq.tile([P, 512], f32, tag="pq")
                pqs = [pq0, pq1, pq2, pq3]
                for d in range(ND):
                    for k in range(4):
                        i = 4 * g + k
                        tsl = slice(i * P, (i + 1) * P)
                        nc.tensor.matmul(pqs[k][:], xT_sb[:, d, tsl],
                                         wqkv_sb[:, d],
                                         start=(d == 0), stop=(d == ND - 1))
                    emit_sig(1)
                for k in range(4):
                    i = 4 * g + k
                    tsl = slice(i * P, (i + 1) * P)
                    pq = pqs[k]
                    nc.scalar.copy(v_sb[:, i, 0:D_HEAD], pq[:, 384:512])
                    qks = work.tile([P, 3, D_HEAD], f32, tag="qks")
                    nc.scalar.copy(qks[:], pq[:, 0:3 * D_HEAD])
                    rot = work.tile([P, 3, D_HEAD], f32, tag="rot")
                    nc.gpsimd.tensor_scalar_mul(rot[:, :, 0:64],
                                                qks[:, :, 64:128], -1.0)
                    nc.gpsimd.tensor_copy(rot[:, :, 64:128], qks[:, :, 0:64])
                    ca = cs_sb[:, i, 0:D_HEAD]
                    cos3 = bass.AP(tensor=ca.tensor, offset=ca.offset,
                                   ap=[ca.ap[0], [0, 3], [1, D_HEAD]])
                    sa = cs_sb[:, i, D_HEAD:2 * D_HEAD]
                    sin3 = bass.AP(tensor=sa.tensor, offset=sa.offset,
                                   ap=[sa.ap[0], [0, 3], [1, D_HEAD]])
                    m1 = work.tile([P, 3, D_HEAD], f32, tag="m1")
                    nc.vector.tensor_mul(m1[:], qks[:], cos3)
                    m2 = work.tile([P, 3, D_HEAD], f32, tag="m2")
                    nc.vector.tensor_mul(m2[:], rot[:], sin3)
                    roped = work.tile([P, 3, D_HEAD], bf16, tag="roped")
                    nc.vector.tensor_add(roped[:], m1[:], m2[:])
                    if rope_pending:
                        emit_rope_t(*rope_pending.pop(0))
                    rope_pending.append((tsl, roped))
            emit_sig(len(sig_tasks) - sig_pos[0])
            for pnd in rope_pending:
                emit_rope_t(*pnd)
            rope_pending.clear()

        # ---------------- attention ----------------
        with tc.tile_pool(name="psS", bufs=2, space="PSUM") as psS, \
             tc.tile_pool(name="psAV", bufs=2, space="PSUM") as psAV, \
             tc.tile_pool(name="psT", bufs=1, space="PSUM") as psT, \
             tc.tile_pool(name="psPO", bufs=1, space="PSUM") as psPO, \
             tc.tile_pool(name="pmp", bufs=6) as pmp, \
             tc.tile_pool(name="outp", bufs=4) as outp:
            pending = []

            def emit_row_tail(ri, risl, raosbs):
                for h in range(HPC):
                    pt = psT.tile([P, P], bf16, tag="pt")
                    nc.tensor.transpose(pt[:], raosbs[h][:], ident[:])
                    nc.vector.tensor_copy(aoutT[:, h, risl], pt[:])
                for mch in range(D_MODEL // 512):
                    po = psPO.tile([P, 512], f32, tag="po")
                    for h in range(HPC):
                        nc.tensor.matmul(po[:], aoutT[:, h, risl],
                                         wo_sb[:, h, mch * 512:(mch + 1) * 512],
                                         start=(h == 0), stop=(h == HPC - 1))
                    ob = outp.tile([P, 512], f32, tag="ob")
                    if mch % 2 == 0:
                        nc.scalar.copy(ob[:], po[:])
                    else:
                        nc.vector.tensor_copy(ob[:], po[:])
                    nc.sync.dma_start(
                        out=out[risl, mch * 512:(mch + 1) * 512], in_=ob[:])

            for i in range(NT):
                isl = slice(i * P, (i + 1) * P)
                base = i * (i + 1) // 2
                avrs0 = psAV.tile([P, P + 1], f32, tag="avrs")
                avrs1 = psAV.tile([P, P + 1], f32, tag="avrs")
                avrs = [avrs0, avrs1]
                for g0 in range(0, i + 1, JG):
                    gn = min(JG, i + 1 - g0)
                    spg = psS.tile([P, JG, HPC * P], f32, tag="spg")
                    for k in range(gn):
                        jt = g0 + k
                        jsl = slice(jt * P, (jt + 1) * P)
                        nc.tensor.matmul(spg[:, k, :], kt_sb[:, jsl],
                                         qt_sb[:, :, isl],
                                         start=True, stop=False)
                        for h in range(HPC):
                            nc.tensor.matmul(spg[:, k, h * P:(h + 1) * P],
                                             diagI[:, h], msig[:, base + jt, :],
                                             start=False, stop=False)
                        # decay bias c_h*(j - K_i): rank-1 over the 6-row
                        # hi/mid/lo-per-head decomposition
                        nc.tensor.matmul(spg[:, k, :], cj_sb[0:KC, i - jt, :],
                                         sel[0:KC, :],
                                         start=False, stop=True)
                    pmg = pmp.tile([P, HPC, JG, P], bf16, tag="pmg")
                    for h in range(HPC):
                        nc.scalar.activation(pmg[:, h, 0:gn, :],
                                             spg[:, 0:gn, h * P:(h + 1) * P],
                                             AF.Exp)
                        if g0 + gn - 1 == i:
                            # causal diag tile: keep ti (free) >= j, else 0
                            nc.gpsimd.affine_select(
                                out=pmg[:, h, gn - 1, :],
                                in_=pmg[:, h, gn - 1, :], pattern=[[1, P]],
                                compare_op=ALU.is_ge, fill=0.0,
                                base=0, channel_multiplier=-1)
                        for k in range(gn):
                            jt = g0 + k
                            nc.tensor.matmul(avrs[h][:, 0:P + 1],
                                             pmg[:, h, k, :], v_sb[:, jt],
                                             start=(jt == 0), stop=(jt == i))
                # reciprocal + scale on DVE; PE-side transpose + out-proj
                # are DEFERRED one row so the in-order PE queue never stalls
                # on this DVE chain
                aosbs = []
                for h in range(HPC):
                    rinv = small.tile([P, 1], f32, tag="rinv")
                    nc.vector.reciprocal(rinv[:], avrs[h][:, P:P + 1])
                    aosb = work.tile([P, P], bf16, tag="aosb")
                    nc.vector.tensor_scalar_mul(aosb[:], avrs[h][:, 0:P], rinv[:])
                    aosbs.append(aosb)
                if pending:
                    emit_row_tail(*pending[0])
                    pending.clear()
                pending.append((i, isl, aosbs))
            emit_row_tail(*pending[0])
    nc.compile()
    return nc


_NC_CACHE = None


def kernel(**inputs):
    global _NC_CACHE
    x = np.asarray(inputs["x"])
    Wq = np.asarray(inputs["Wq"]); Wk = np.asarray(inputs["Wk"])
    Wv = np.asarray(inputs["Wv"]); Wo = np.asarray(inputs["Wo"])
    pol_dir = np.asarray(inputs["pol_dir"]); pol_WA = np.asarray(inputs["pol_WA"])
    pol_WB = np.asarray(inputs["pol_WB"]); pol_gate = np.asarray(inputs["pol_gate"])
    gtp_gamma = np.asarray(inputs["gtp_gamma"])

    import ml_dtypes
    bf = ml_dtypes.bfloat16
    assert x.shape == (1, T, D_MODEL)

    pol = np.clip(pol_dir.astype(np.float64), -1.0, 1.0)
    gamma = np.maximum(np.log1p(np.exp(gtp_gamma.astype(np.float64))), 1e-6)
    c_h = (pol / float(MAX_SEQ_LEN) + gamma).astype(np.float64)
    gate = (1.0 / (1.0 + np.exp(-pol_gate.astype(np.float64)))).astype(np.float32)

    inv_freq = 1.0 / (ROPE_BASE ** (np.arange(0, D_HEAD, 2, dtype=np.float64) / D_HEAD))
    ang = np.arange(T, dtype=np.float64)[:, None] * inv_freq[None, :]
    cosf = np.concatenate([np.cos(ang), np.cos(ang)], 1)
    sinf = np.concatenate([np.sin(ang), np.sin(ang)], 1)
    csf = np.concatenate([cosf, sinf], 1).astype(bf)   # [T, 256]

    xT = np.ascontiguousarray(x[0].T).astype(bf)
    inv_sqrt_d = 1.0 / np.sqrt(float(D_HEAD))

    # cj[p, d] = c_h * (p - 127 - 128*d) as 3-term bf16 decomposition
    pcol = np.arange(P, dtype=np.float64)[None, :]          # [1, P] (j within tile)
    dgrid = np.arange(NT, dtype=np.float64)[:, None]        # [NT, 1]
    off = pcol - 127.0 - 128.0 * dgrid                      # [NT, P]

    sel_np = np.zeros((3 * HPC, HPC * P), dtype=bf)
    for h in range(HPC):
        sel_np[3 * h:3 * h + 3, h * P:(h + 1) * P] = bf(1.0)

    in_maps = []
    for c in range(N_CORES):
        hs = slice(2 * c * D_HEAD, (2 * c + 2) * D_HEAD)
        kvh = c // 2
        kv = slice(kvh * D_HEAD, (kvh + 1) * D_HEAD)
        wqkv_np = np.concatenate(
            [Wq[:, hs] * inv_sqrt_d, Wk[:, kv], Wv[:, kv]], axis=1)
        cjtab_np = np.zeros((3 * HPC, NT * P), dtype=bf)
        for h in range(HPC):
            val = (c_h[2 * c + h] * off).reshape(-1)        # [NT*P]
            rem = val.copy()
            for kk in range(3):
                comp = rem.astype(bf)
                cjtab_np[3 * h + kk] = comp
                rem = rem - comp.astype(np.float64)
        in_maps.append({
            "xT": xT,
            "wqkv": np.ascontiguousarray(wqkv_np).astype(bf),
            "wab": np.ascontiguousarray(
                np.concatenate([pol_WA, pol_WB], axis=1)).astype(bf),
            "wo": np.ascontiguousarray(Wo[hs, :]).astype(bf),
            "cs": csf,
            "hpar": gate[None, 2 * c:2 * c + 2].astype(np.float32),
            "cjtab": cjtab_np,
            "seld": sel_np,
        })

    if _NC_CACHE is None:
        _NC_CACHE = _build_kernel()
    from concourse.bass_utils import run_bass_kernel_spmd
    res = run_bass_kernel_spmd(_NC_CACHE, in_maps, core_ids=list(range(N_CORES)))
    total = np.zeros((T, D_MODEL), dtype=np.float32)
    for c in range(N_CORES):
        total += res.results[c]["out"]
    return total[None, :, :]
